# revision 1
# baseline (speedup 1.0000x reference)
"""Single-head attention (B=16, N=2048, d_in=256, d_qk=d_v=64) on 8 TRN2
NeuronCores, data-parallel over batch (2 batches per core, no collectives).

Math per batch b:
  q = x@wq + bq ; k = x@wk + bk ; v = x@wv + bv
  out = softmax(q k^T / 8) v

Device layout choices:
  - host feeds x^T (d on partitions) so every matmul has its contraction
    dim on partitions with zero on-device transposes
  - scores are computed TRANSPOSED: ST[m(keys) partitions, n(queries) free]
    so that P = exp(ST) is directly the rhs of the attention@V matmul
    (lhsT = V[m, dv]); exp needs no max-subtraction (scores sigma~0.6)
  - wv is augmented with a 65th column of ones (via the bias row) so the
    softmax denominator appears as row 64 of the output accumulator
  - denominator reciprocal is broadcast across partitions with a ones
    matmul; final output is written as out^T [dv, n] and transposed on host
"""

import os
from contextlib import ExitStack

import numpy as np

N_CORES = 8
B, N, D_IN, D_QK, D_V = 16, 2048, 256, 64, 64
BPC = B // N_CORES  # batches per core
EV1 = D_V + 1  # v augmented with ones column (softmax denominator)
NH = 1024  # n (query) half: ST psum tile is [128, NH] = 2 banks

_CACHE = {}

# exec time of the most recent profiled run (test harness convenience)
LAST_EXEC_TIME_NS = None


def _build_nc():
    import concourse.tile as tile
    from concourse import bacc, mybir

    f32 = mybir.dt.float32
    bf16 = mybir.dt.bfloat16
    Exp = mybir.ActivationFunctionType.Exp

    nc = bacc.Bacc(
        "TRN2", target_bir_lowering=False, debug=False,
        enable_asserts=True, num_devices=N_CORES,
    )

    xt = nc.dram_tensor("xt", [BPC, D_IN, N], bf16, kind="ExternalInput").ap()
    wq = nc.dram_tensor("wq", [D_IN, D_QK], bf16, kind="ExternalInput").ap()
    wk = nc.dram_tensor("wk", [D_IN, D_QK], bf16, kind="ExternalInput").ap()
    wv = nc.dram_tensor("wv", [D_IN, EV1], bf16, kind="ExternalInput").ap()
    bq = nc.dram_tensor("bq", [D_QK, 1], f32, kind="ExternalInput").ap()
    bk = nc.dram_tensor("bk", [D_QK, 1], f32, kind="ExternalInput").ap()
    bv = nc.dram_tensor("bv", [128, EV1], f32, kind="ExternalInput").ap()
    ones = nc.dram_tensor("ones", [1, D_V], bf16, kind="ExternalInput").ap()
    out = nc.dram_tensor("out", [BPC, D_V, N], f32, kind="ExternalOutput").ap()

    with tile.TileContext(nc) as tc, ExitStack() as ctx:
        consts = ctx.enter_context(tc.tile_pool(name="consts", bufs=1))
        xt_pool = ctx.enter_context(tc.tile_pool(name="xt", bufs=2))
        qk_pool = ctx.enter_context(tc.tile_pool(name="qk", bufs=2))
        v_pool = ctx.enter_context(tc.tile_pool(name="v", bufs=2))
        p_pool = ctx.enter_context(tc.tile_pool(name="p", bufs=3))
        rb_pool = ctx.enter_context(tc.tile_pool(name="rb", bufs=2))
        small = ctx.enter_context(tc.tile_pool(name="small", bufs=2))
        outp = ctx.enter_context(tc.tile_pool(name="outp", bufs=3))
        mm_ps = ctx.enter_context(tc.tile_pool(name="mm_ps", bufs=2, space="PSUM"))
        st_ps = ctx.enter_context(tc.tile_pool(name="st_ps", bufs=2, space="PSUM"))
        ot_ps = ctx.enter_context(tc.tile_pool(name="ot_ps", bufs=1, space="PSUM"))

        with nc.allow_low_precision(reason="bf16 attention intermediates"):
            # ---- constants (loaded once) ----
            wq_sb = consts.tile([128, 2 * D_QK], bf16, tag="wq")
            wk_sb = consts.tile([128, 2 * D_QK], bf16, tag="wk")
            wv_sb = consts.tile([128, 2 * EV1], bf16, tag="wv")
            for kk in range(2):
                nc.sync.dma_start(
                    wq_sb[:, kk * D_QK:(kk + 1) * D_QK], wq[kk * 128:(kk + 1) * 128, :])
                nc.sync.dma_start(
                    wk_sb[:, kk * D_QK:(kk + 1) * D_QK], wk[kk * 128:(kk + 1) * 128, :])
                nc.sync.dma_start(
                    wv_sb[:, kk * EV1:(kk + 1) * EV1], wv[kk * 128:(kk + 1) * 128, :])
            bq_sb = consts.tile([D_QK, 1], f32, tag="bq")
            bk_sb = consts.tile([D_QK, 1], f32, tag="bk")
            bv_sb = consts.tile([128, EV1], f32, tag="bv")
            ones_sb = consts.tile([1, D_V], bf16, tag="ones")
            nc.sync.dma_start(bq_sb[:], bq[:, :])
            nc.sync.dma_start(bk_sb[:], bk[:, :])
            nc.sync.dma_start(bv_sb[:], bv[:, :])
            nc.sync.dma_start(ones_sb[:], ones[:, :])

            for b in range(BPC):
                # ---- load x^T (two 128-row d-tiles) ----
                xt_sb = xt_pool.tile([128, 2 * N], bf16, tag="xt")
                for kk in range(2):
                    for hh in range(2):
                        nc.sync.dma_start(
                            xt_sb[:, kk * N + hh * NH: kk * N + (hh + 1) * NH],
                            xt[b, kk * 128:(kk + 1) * 128, hh * NH:(hh + 1) * NH])

                # ---- QT = (wq/8)^T x^T + bq/8 ; KT likewise  [64, N] bf16 ----
                qt_sb = qk_pool.tile([D_QK, N], bf16, tag="qt")
                kt_sb = qk_pool.tile([D_QK, N], bf16, tag="kt")
                for w_sb, b_sb, dst in ((wq_sb, bq_sb, qt_sb), (wk_sb, bk_sb, kt_sb)):
                    for j in range(N // 512):
                        ps = mm_ps.tile([D_QK, 512], f32, tag="mm")
                        for kk in range(2):
                            nc.tensor.matmul(
                                ps[:],
                                w_sb[:, kk * D_QK:(kk + 1) * D_QK],
                                xt_sb[:, kk * N + j * 512: kk * N + (j + 1) * 512],
                                start=(kk == 0), stop=(kk == 1))
                        nc.vector.tensor_scalar_add(
                            dst[:, j * 512:(j + 1) * 512], ps[:], b_sb[:])

                # ---- V_aug = x wv_aug + bv_aug  [m 128, 65] bf16, 16 tiles ----
                v_sb = v_pool.tile([128, 16 * EV1], bf16, tag="v")
                for m in range(16):
                    ps = mm_ps.tile([128, EV1], f32, tag="mm")
                    for kk in range(2):
                        nc.tensor.matmul(
                            ps[:],
                            xt_sb[:, kk * N + m * 128: kk * N + (m + 1) * 128],
                            wv_sb[:, kk * EV1:(kk + 1) * EV1],
                            start=(kk == 0), stop=(kk == 1))
                    nc.vector.tensor_add(
                        v_sb[:, m * EV1:(m + 1) * EV1], ps[:], bv_sb[:])

                # ---- attention, per query-half ----
                for h in range(N // NH):
                    ot = ot_ps.tile([EV1, NH], f32, tag="ot")
                    for m in range(16):
                        st = st_ps.tile([128, NH], f32, tag="st")
                        for j in range(NH // 512):
                            nc.tensor.matmul(
                                st[:, j * 512:(j + 1) * 512],
                                kt_sb[:, m * 128:(m + 1) * 128],
                                qt_sb[:, h * NH + j * 512: h * NH + (j + 1) * 512],
                                start=True, stop=True)
                        p = p_pool.tile([128, NH], bf16, tag="p")
                        nc.scalar.activation(p[:], st[:], Exp)
                        for j in range(NH // 512):
                            nc.tensor.matmul(
                                ot[:, j * 512:(j + 1) * 512],
                                v_sb[:, m * EV1:(m + 1) * EV1],
                                p[:, j * 512:(j + 1) * 512],
                                start=(m == 0), stop=(m == 15),
                                skip_group_check=True)

                    # ---- epilogue: divide by denominator (row 64), store ----
                    rcp = small.tile([1, NH], f32, tag="rcp")
                    nc.vector.reciprocal(rcp[:], ot[D_V:EV1, :])
                    rcp16 = small.tile([1, NH], bf16, tag="rcp16")
                    nc.vector.tensor_copy(rcp16[:], rcp[:])
                    rb_sb = rb_pool.tile([D_V, NH], bf16, tag="rb")
                    for j in range(NH // 512):
                        rb_ps = mm_ps.tile([D_V, 512], f32, tag="mm")
                        nc.tensor.matmul(
                            rb_ps[:], ones_sb[:],
                            rcp16[:, j * 512:(j + 1) * 512],
                            start=True, stop=True)
                        nc.vector.tensor_copy(rb_sb[:, j * 512:(j + 1) * 512], rb_ps[:])
                    o_sb = outp.tile([D_V, NH], f32, tag="o")
                    nc.vector.tensor_mul(o_sb[:], ot[0:D_V, :], rb_sb[:])
                    nc.sync.dma_start(out[b, :, h * NH:(h + 1) * NH], o_sb[:])

    nc.compile()
    return nc


def _get_nc():
    if "nc" not in _CACHE:
        _CACHE["nc"] = _build_nc()
    return _CACHE["nc"]


def kernel(x, wq, bq, wk, bk, wv, bv):
    global LAST_EXEC_TIME_NS
    import ml_dtypes
    from concourse.bass_utils import run_bass_kernel_spmd

    bf16 = ml_dtypes.bfloat16
    x = np.asarray(x, dtype=np.float32)
    wq = np.asarray(wq, dtype=np.float32)
    wk = np.asarray(wk, dtype=np.float32)
    wv = np.asarray(wv, dtype=np.float32)
    bq = np.asarray(bq, dtype=np.float32)
    bk = np.asarray(bk, dtype=np.float32)
    bv = np.asarray(bv, dtype=np.float32)

    # host-side layout/precision prep
    xt = np.ascontiguousarray(x.transpose(0, 2, 1)).astype(bf16)  # [B, D, N]
    wq8 = (wq / 8.0).astype(bf16)  # fold 1/sqrt(d_qk) into q projection
    bq8 = (bq / 8.0).astype(np.float32).reshape(D_QK, 1)
    wkb = wk.astype(bf16)
    bkb = bk.astype(np.float32).reshape(D_QK, 1)
    wv_aug = np.zeros((D_IN, EV1), np.float32)
    wv_aug[:, :D_V] = wv
    wv_augb = wv_aug.astype(bf16)
    bv_aug = np.zeros((128, EV1), np.float32)
    bv_aug[:, :D_V] = bv  # broadcast bias to all partitions
    bv_aug[:, D_V] = 1.0  # ones column -> softmax denominator
    ones = np.ones((1, D_V), bf16)

    in_maps = []
    for c in range(N_CORES):
        in_maps.append({
            "xt": np.ascontiguousarray(xt[BPC * c: BPC * (c + 1)]),
            "wq": wq8, "wk": wkb, "wv": wv_augb,
            "bq": bq8, "bk": bkb, "bv": bv_aug, "ones": ones,
        })

    nc = _get_nc()
    trace = bool(int(os.environ.get("ATTN_PROFILE", "0")))
    res = run_bass_kernel_spmd(
        nc, in_maps, core_ids=list(range(N_CORES)), trace=trace)
    LAST_EXEC_TIME_NS = res.exec_time_ns

    outs = np.stack([r["out"] for r in res.results])  # [8, BPC, 64, N]
    out = outs.reshape(B, D_V, N).transpose(0, 2, 1)  # [B, N, 64]
    return np.ascontiguousarray(out).astype(np.float32)


# revision 8
# speedup vs baseline: 1.2479x; 1.2479x over previous
"""Single-head attention (B=16, N=2048, d_in=256, d_qk=d_v=64) on 8 TRN2
NeuronCores, data-parallel over batch (2 batches per core, no collectives).

Math per batch b:
  q = x@wq + bq ; k = x@wk + bk ; v = x@wv + bv
  out = softmax(q k^T / 8) v

Device layout choices:
  - host feeds x^T (d on partitions) so every matmul has its contraction
    dim on partitions with zero on-device transposes
  - scores are computed TRANSPOSED: ST[m(keys) partitions, n(queries) free]
    so that P = exp(ST) is directly the rhs of the attention@V matmul
    (lhsT = V[m, dv]); exp needs no max-subtraction (scores sigma~0.6)
  - score matmuls have K=64 so PAIRS of key-chunks are row-packed into the
    two halves of the 128x128 PE array (tile_position via base partition);
    Q^T/K^T are duplicated into both partition halves to support this
  - wv is augmented with a 65th column of ones (via the bias row) so the
    softmax denominator appears as row 64 of the output accumulator
  - denominator reciprocal is broadcast across partitions with a ones
    matmul; final output is written as out^T [dv, n] and transposed on host
"""

import os
from contextlib import ExitStack

import numpy as np

N_CORES = 8
B, N, D_IN, D_QK, D_V = 16, 2048, 256, 64, 64
BPC = B // N_CORES  # batches per core
EV1 = D_V + 1  # v augmented with ones column (softmax denominator)
NH = 1024  # query-dim width of an ot psum tile (2 banks)

_CACHE = {}

# exec time of the most recent profiled run (test harness convenience)
LAST_EXEC_TIME_NS = None


def _build_nc():
    import concourse.tile as tile
    from concourse import bacc, mybir

    f32 = mybir.dt.float32
    bf16 = mybir.dt.bfloat16
    Exp = mybir.ActivationFunctionType.Exp

    nc = bacc.Bacc(
        "TRN2", target_bir_lowering=False, debug=False,
        enable_asserts=True, num_devices=N_CORES,
    )

    xt = nc.dram_tensor("xt", [BPC, D_IN, N], bf16, kind="ExternalInput").ap()
    wq = nc.dram_tensor("wq", [D_IN, D_QK], bf16, kind="ExternalInput").ap()
    wk = nc.dram_tensor("wk", [D_IN, D_QK], bf16, kind="ExternalInput").ap()
    wv = nc.dram_tensor("wv", [D_IN, EV1], bf16, kind="ExternalInput").ap()
    bq = nc.dram_tensor("bq", [D_QK, 1], f32, kind="ExternalInput").ap()
    bk = nc.dram_tensor("bk", [D_QK, 1], f32, kind="ExternalInput").ap()
    bv = nc.dram_tensor("bv", [128, EV1], f32, kind="ExternalInput").ap()
    ones = nc.dram_tensor("ones", [1, D_V], bf16, kind="ExternalInput").ap()
    out = nc.dram_tensor("out", [BPC, D_V, N], f32, kind="ExternalOutput").ap()
    den = nc.dram_tensor("den", [BPC, 1, N], f32, kind="ExternalOutput").ap()

    with tile.TileContext(nc) as tc, ExitStack() as ctx:
        consts = ctx.enter_context(tc.tile_pool(name="consts", bufs=1))
        xt_pool = ctx.enter_context(tc.tile_pool(name="xt", bufs=2))
        qk_pool = ctx.enter_context(tc.tile_pool(name="qk", bufs=2))
        v_pool = ctx.enter_context(tc.tile_pool(name="v", bufs=2))
        p_pool = ctx.enter_context(tc.tile_pool(name="p", bufs=4))
        rb_pool = ctx.enter_context(tc.tile_pool(name="rb", bufs=2))
        small = ctx.enter_context(tc.tile_pool(name="small", bufs=2))
        outp = ctx.enter_context(tc.tile_pool(name="outp", bufs=3))
        # one shared PSUM pool: 4 slots x 2 banks = all 8 banks
        psum = ctx.enter_context(tc.tile_pool(name="psum", bufs=4, space="PSUM"))

        with nc.allow_low_precision(reason="bf16 attention intermediates"):
            # ---- constants (loaded once) ----
            wq_sb = consts.tile([128, 2 * D_QK], bf16, tag="wq")
            wk_sb = consts.tile([128, 2 * D_QK], bf16, tag="wk")
            wv_sb = consts.tile([128, 2 * EV1], bf16, tag="wv")
            for kk in range(2):
                nc.sync.dma_start(
                    wq_sb[:, kk * D_QK:(kk + 1) * D_QK], wq[kk * 128:(kk + 1) * 128, :])
                nc.sync.dma_start(
                    wk_sb[:, kk * D_QK:(kk + 1) * D_QK], wk[kk * 128:(kk + 1) * 128, :])
                nc.sync.dma_start(
                    wv_sb[:, kk * EV1:(kk + 1) * EV1], wv[kk * 128:(kk + 1) * 128, :])
            bq_sb = consts.tile([D_QK, 1], f32, tag="bq")
            bk_sb = consts.tile([D_QK, 1], f32, tag="bk")
            bv_sb = consts.tile([128, EV1], f32, tag="bv")
            ones_sb = consts.tile([1, D_V], bf16, tag="ones")
            nc.sync.dma_start(bq_sb[:], bq[:, :])
            nc.sync.dma_start(bk_sb[:], bk[:, :])
            nc.sync.dma_start(bv_sb[:], bv[:, :])
            nc.sync.dma_start(ones_sb[:], ones[:, :])

            for b in range(BPC):
                # ---- load x^T (two 128-row d-tiles, split for early start) ----
                xt_sb = xt_pool.tile([128, 2 * N], bf16, tag="xt")
                for kk in range(2):
                    for hh in range(2):
                        nc.sync.dma_start(
                            xt_sb[:, kk * N + hh * NH: kk * N + (hh + 1) * NH],
                            xt[b, kk * 128:(kk + 1) * 128, hh * NH:(hh + 1) * NH])

                # ---- QT/KT = w^T x^T + bias, duplicated into both halves ----
                qtd = qk_pool.tile([128, N], bf16, tag="qt")
                ktd = qk_pool.tile([128, N], bf16, tag="kt")
                for w_sb, b_sb, dst in ((wq_sb, bq_sb, qtd), (wk_sb, bk_sb, ktd)):
                    for j in range(N // 512):
                        ps = psum.tile([D_QK, 512], f32, tag="big")
                        for kk in range(2):
                            nc.tensor.matmul(
                                ps[:],
                                w_sb[:, kk * D_QK:(kk + 1) * D_QK],
                                xt_sb[:, kk * N + j * 512: kk * N + (j + 1) * 512],
                                start=(kk == 0), stop=(kk == 1))
                        nc.vector.tensor_scalar_add(
                            dst[0:D_QK, j * 512:(j + 1) * 512], ps[:], b_sb[:])
                        # duplicate into partitions 64..127 (for row-packing)
                        nc.sync.dma_start(
                            dst[D_QK:128, j * 512:(j + 1) * 512],
                            dst[0:D_QK, j * 512:(j + 1) * 512])

                # ---- V_aug = x wv_aug + bv_aug  [m 128, 65] bf16, 16 tiles ----
                v_sb = v_pool.tile([128, 16 * EV1], bf16, tag="v")
                for m in range(16):
                    ps = psum.tile([128, EV1], f32, tag="big")
                    for kk in range(2):
                        nc.tensor.matmul(
                            ps[:],
                            xt_sb[:, kk * N + m * 128: kk * N + (m + 1) * 128],
                            wv_sb[:, kk * EV1:(kk + 1) * EV1],
                            start=(kk == 0), stop=(kk == 1))
                    nc.vector.tensor_add(
                        v_sb[:, m * EV1:(m + 1) * EV1], ps[:], bv_sb[:])

                # ---- attention, per query-half of 1024 ----
                for h in range(N // NH):
                    ot = psum.tile([EV1, NH], f32, tag="big")
                    for mp in range(8):  # key-chunk pair (2*mp, 2*mp+1)
                        m0, m1 = 2 * mp, 2 * mp + 1
                        for js in range(2):  # 512-wide query slice in half
                            q0 = h * NH + js * 512
                            st = psum.tile([128, NH], f32, tag="big")
                            # row-packed pair: array rows 0-63 and 64-127
                            nc.tensor.matmul(
                                st[:, 0:512],
                                ktd[0:D_QK, m0 * 128:(m0 + 1) * 128],
                                qtd[0:D_QK, q0:q0 + 512],
                                start=True, stop=True)
                            nc.tensor.matmul(
                                st[:, 512:1024],
                                ktd[D_QK:128, m1 * 128:(m1 + 1) * 128],
                                qtd[D_QK:128, q0:q0 + 512],
                                start=True, stop=True)
                            p = p_pool.tile([128, NH], bf16, tag="p")
                            nc.scalar.activation(p[:], st[:], Exp)
                            nc.tensor.matmul(
                                ot[:, js * 512:(js + 1) * 512],
                                v_sb[:, m0 * EV1:(m0 + 1) * EV1],
                                p[:, 0:512],
                                start=(mp == 0), stop=False,
                                skip_group_check=True)
                            nc.tensor.matmul(
                                ot[:, js * 512:(js + 1) * 512],
                                v_sb[:, m1 * EV1:(m1 + 1) * EV1],
                                p[:, 512:1024],
                                start=False, stop=(mp == 7),
                                skip_group_check=True)

                    # ---- epilogue: divide by denominator (row 64), store ----
                    den_sb = small.tile([1, NH], f32, tag="den")
                    nc.vector.tensor_copy(den_sb[:], ot[D_V:EV1, :])
                    nc.sync.dma_start(den[b, :, h * NH:(h + 1) * NH], den_sb[:, :])
                    rcp = small.tile([1, NH], f32, tag="rcp")
                    nc.vector.reciprocal_approx_fast(rcp[:], den_sb[:])
                    rcp16 = small.tile([1, NH], bf16, tag="rcp16")
                    nc.vector.tensor_copy(rcp16[:], rcp[:])
                    rb_sb = rb_pool.tile([D_V, NH], bf16, tag="rb")
                    for j in range(NH // 512):
                        rb_ps = psum.tile([D_V, 512], f32, tag="big")
                        nc.tensor.matmul(
                            rb_ps[:], ones_sb[:],
                            rcp16[:, j * 512:(j + 1) * 512],
                            start=True, stop=True)
                        nc.vector.tensor_copy(rb_sb[:, j * 512:(j + 1) * 512], rb_ps[:])
                    o_sb = outp.tile([D_V, NH], f32, tag="o")
                    nc.vector.tensor_mul(o_sb[:], ot[0:D_V, :], rb_sb[:])
                    nc.sync.dma_start(out[b, :, h * NH:(h + 1) * NH], o_sb[:])

    nc.compile()
    return nc


def _get_nc():
    if "nc" not in _CACHE:
        _CACHE["nc"] = _build_nc()
    return _CACHE["nc"]


def kernel(x, wq, bq, wk, bk, wv, bv):
    global LAST_EXEC_TIME_NS
    import ml_dtypes
    from concourse.bass_utils import run_bass_kernel_spmd

    bf16 = ml_dtypes.bfloat16
    x = np.asarray(x, dtype=np.float32)
    wq = np.asarray(wq, dtype=np.float32)
    wk = np.asarray(wk, dtype=np.float32)
    wv = np.asarray(wv, dtype=np.float32)
    bq = np.asarray(bq, dtype=np.float32)
    bk = np.asarray(bk, dtype=np.float32)
    bv = np.asarray(bv, dtype=np.float32)

    # host-side layout/precision prep
    xt = np.ascontiguousarray(x.transpose(0, 2, 1)).astype(bf16)  # [B, D, N]
    wq8 = (wq / 8.0).astype(bf16)  # fold 1/sqrt(d_qk) into q projection
    bq8 = (bq / 8.0).astype(np.float32).reshape(D_QK, 1)
    wkb = wk.astype(bf16)
    bkb = bk.astype(np.float32).reshape(D_QK, 1)
    wv_aug = np.zeros((D_IN, EV1), np.float32)
    wv_aug[:, :D_V] = wv
    wv_augb = wv_aug.astype(bf16)
    bv_aug = np.zeros((128, EV1), np.float32)
    bv_aug[:, :D_V] = bv  # broadcast bias to all partitions
    bv_aug[:, D_V] = 1.0  # ones column -> softmax denominator
    ones = np.ones((1, D_V), bf16)

    in_maps = []
    for c in range(N_CORES):
        in_maps.append({
            "xt": np.ascontiguousarray(xt[BPC * c: BPC * (c + 1)]),
            "wq": wq8, "wk": wkb, "wv": wv_augb,
            "bq": bq8, "bk": bkb, "bv": bv_aug, "ones": ones,
        })

    nc = _get_nc()
    trace = bool(int(os.environ.get("ATTN_PROFILE", "0")))
    res = run_bass_kernel_spmd(
        nc, in_maps, core_ids=list(range(N_CORES)), trace=trace)
    LAST_EXEC_TIME_NS = res.exec_time_ns

    outs = np.stack([r["out"] for r in res.results])  # [8, BPC, 64, N]
    out = outs.reshape(B, D_V, N).transpose(0, 2, 1)  # [B, N, 64]
    return np.ascontiguousarray(out).astype(np.float32)
